# revision 8
# baseline (speedup 1.0000x reference)
"""Trainium2 Bass kernel for a 3-layer binarized MLP with train-mode BatchNorm.

Network (B=16384, IN=4096, H1=256, H2=128, NC=12):
    h1 = x @ sign(W1).T [+ sign(b1)]        <- bias cancels in train-mode BN
    h1 = clip(BN_train(h1; g1, be1), -1, 1)
    h2 = h1 @ sign(W2).T [+ sign(b2)]       <- bias cancels in train-mode BN
    h2 = clip(BN_train(h2; g2, be2), -1, 1)
    out = h2 @ W4.T + b4

Strategy: pure data-parallel over 8 NeuronCores (2048 batch rows each).
BN statistics are over the FULL batch, so each core computes per-feature
partial (mean, var) via hw bn_stats/bn_aggr and the cores AllReduce a tiny
(mean, var+mean^2) payload between layers.  Everything is computed in a
feature-major layout (features on partitions, batch on the free axis) so
the BN affine is a per-partition scale/bias and batch reductions are
free-axis reductions.

Host-side prep (cheap, linear): fold sign() into the weights, pre-transpose
x and weights so all DMA is contiguous, cast matmul operands to bf16
(measured end-to-end rel-l2 error vs the fp32 reference: ~2.4e-3).
"""

import os
import sys
import types

import numpy as np
import ml_dtypes

B, IN, H1, H2, NCOUT = 16384, 4096, 256, 128, 12
N_CORES = 8
BS = B // N_CORES            # 2048 batch rows per core
NBLK = BS // 512             # 4 psum blocks of 512 along batch
KC = IN // 128               # 32 contraction chunks, layer 1
M1 = H1 // 128               # 2 feature chunks, layer 1 output
GK = 4                       # max k-chunks per x DMA group (2 MB per DMA)
GROUPS = [2, 2] + [4] * 7    # k-chunk group sizes (small first groups
                             # so the first matmuls start early)
assert sum(GROUPS) == KC
EPS = 1e-5

bf16 = ml_dtypes.bfloat16

_COMPILED = {}
LAST_EXEC_NS = None


def _install_ntff_hook():
    """Best-effort: register the axon NTFF profile hook that the boot script
    could not (antenv.axon_hooks is absent from this image)."""
    if "antenv.axon_hooks" in sys.modules:
        return
    try:
        from trn_agent_boot.trn_boot import _ntff_profile_via_ctypes

        hook = _ntff_profile_via_ctypes("/opt/axon/libaxon_pjrt.so")
        mod = types.ModuleType("antenv.axon_hooks")
        mod.get_axon_ntff_profile_hook = lambda: hook
        sys.modules["antenv.axon_hooks"] = mod
    except Exception:
        pass


def _build():
    import concourse.bacc as bacc
    import concourse.tile as tile
    import concourse.mybir as mybir

    dt = mybir.dt
    F = mybir.ActivationFunctionType
    A = mybir.AluOpType
    RG = [list(range(N_CORES))]

    nc = bacc.Bacc("TRN2", target_bir_lowering=False, debug=False,
                   num_devices=N_CORES)

    xt = nc.dram_tensor("xt", [IN, BS], dt.bfloat16, kind="ExternalInput").ap()
    w1t = nc.dram_tensor("w1t", [IN, H1], dt.bfloat16, kind="ExternalInput").ap()
    w2t = nc.dram_tensor("w2t", [H1, H2], dt.bfloat16, kind="ExternalInput").ap()
    w4t = nc.dram_tensor("w4t", [H2, NCOUT], dt.bfloat16, kind="ExternalInput").ap()
    g1 = nc.dram_tensor("g1", [H1], dt.float32, kind="ExternalInput").ap()
    be1 = nc.dram_tensor("be1", [H1], dt.float32, kind="ExternalInput").ap()
    g2 = nc.dram_tensor("g2", [H2], dt.float32, kind="ExternalInput").ap()
    be2 = nc.dram_tensor("be2", [H2], dt.float32, kind="ExternalInput").ap()
    b4 = nc.dram_tensor("b4", [NCOUT], dt.float32, kind="ExternalInput").ap()
    out = nc.dram_tensor("out", [NCOUT, BS], dt.float32, kind="ExternalOutput").ap()

    with tile.TileContext(nc) as tc:
        with (
            tc.tile_pool(name="xp", bufs=3) as xp,
            tc.tile_pool(name="wp", bufs=1) as wp,
            tc.tile_pool(name="hp", bufs=1) as hp,
            tc.tile_pool(name="pp", bufs=1, space="PSUM") as pp,
            tc.tile_pool(name="sp", bufs=1) as sp,
            tc.tile_pool(name="scr", bufs=2) as scr,
            tc.tile_pool(name="dp", bufs=1, space="DRAM") as dp,
        ):
            # ---- static tiles -------------------------------------------------
            w2s = wp.tile([128, M1, H2], dt.bfloat16, tag="w2s")
            w4s = wp.tile([H2, NCOUT], dt.bfloat16, tag="w4s")
            g1t = sp.tile([128, M1], dt.float32, tag="g1t")
            be1t = sp.tile([128, M1], dt.float32, tag="be1t")
            g2t = sp.tile([128, 1], dt.float32, tag="g2t")
            be2t = sp.tile([128, 1], dt.float32, tag="be2t")
            b4t = sp.tile([NCOUT, 1], dt.float32, tag="b4t")

            h1f = hp.tile([128, M1, BS], dt.float32, tag="h1f")
            h1c = hp.tile([128, M1, BS], dt.bfloat16, tag="h1c")
            out_sb = hp.tile([NCOUT, BS], dt.float32, tag="out_sb")

            stats1 = sp.tile([128, M1, NBLK, 6], dt.float32, tag="stats1")
            pay1 = sp.tile([128, M1, 2], dt.float32, tag="pay1")
            gath1 = sp.tile([128, M1 * 2, N_CORES], dt.float32, tag="gath1")
            gst1 = sp.tile([128, M1, 2], dt.float32, tag="gst1")
            a1 = sp.tile([128, M1], dt.float32, tag="a1")
            c1 = sp.tile([128, M1], dt.float32, tag="c1")
            t1a = sp.tile([128, M1], dt.float32, tag="t1a")
            t1b = sp.tile([128, M1], dt.float32, tag="t1b")

            stats2 = sp.tile([128, NBLK, 6], dt.float32, tag="stats2")
            pay2 = sp.tile([128, 2], dt.float32, tag="pay2")
            gath2 = sp.tile([128, 2, N_CORES], dt.float32, tag="gath2")
            gst2 = sp.tile([128, 2], dt.float32, tag="gst2")
            a2 = sp.tile([128, 1], dt.float32, tag="a2")
            c2 = sp.tile([128, 1], dt.float32, tag="c2")
            t2a = sp.tile([128, 1], dt.float32, tag="t2a")
            t2b = sp.tile([128, 1], dt.float32, tag="t2b")

            cin1 = dp.tile([128, M1 * 2], dt.float32, tag="cin1")
            cout1 = dp.tile([128 * N_CORES, M1 * 2], dt.float32, tag="cout1")
            cin2 = dp.tile([128, 2], dt.float32, tag="cin2")
            cout2 = dp.tile([128 * N_CORES, 2], dt.float32, tag="cout2")

            # psum accumulators: 8 banks = (M1 x NBLK) tiles of [128, 512] f32
            ps = [[pp.tile([128, 512], dt.float32, tag=f"ps_{m}_{b}",
                           name=f"ps_{m}_{b}")
                   for b in range(NBLK)] for m in range(M1)]

            # ---- weight / param loads (scalar HWDGE queue, parallel to x) ----
            # w1 arrives in per-group pieces so the first matmuls start early
            w1g = []
            coff = 0
            for gi, gk in enumerate(GROUPS):
                t = wp.tile([128, gk, H1], dt.bfloat16, tag=f"w1g_{gi}",
                            name=f"w1g_{gi}")
                nc.scalar.dma_start(
                    out=t[:],
                    in_=w1t[coff * 128:(coff + gk) * 128, :]
                        .rearrange("(c p) m -> p c m", p=128),
                )
                w1g.append(t)
                coff += gk
            nc.scalar.dma_start(out=w2s[:], in_=w2t.rearrange("(c p) m -> p c m", p=128))
            nc.scalar.dma_start(out=w4s[:], in_=w4t[:, :])
            nc.scalar.dma_start(out=g1t[:], in_=g1.rearrange("(m p) -> p m", p=128))
            nc.scalar.dma_start(out=be1t[:], in_=be1.rearrange("(m p) -> p m", p=128))
            nc.scalar.dma_start(out=g2t[:], in_=g2.rearrange("(p o) -> p o", o=1))
            nc.scalar.dma_start(out=be2t[:], in_=be2.rearrange("(p o) -> p o", o=1))
            nc.scalar.dma_start(out=b4t[:], in_=b4.rearrange("(p o) -> p o", o=1))

            # ---- phase 1: h1 = x @ sign(W1).T  (feature-major) ---------------
            coff = 0
            for gi, gk in enumerate(GROUPS):
                xg = xp.tile([128, GK, BS], dt.bfloat16, tag="xg", name="xg")
                xeng = nc.sync if gi % 2 == 0 else nc.scalar
                xeng.dma_start(
                    out=xg[:, 0:gk, :],
                    in_=xt[coff * 128:(coff + gk) * 128, :]
                        .rearrange("(c p) b -> p c b", p=128),
                )
                for ci in range(gk):
                    c = coff + ci
                    for m in range(M1):
                        for b in range(NBLK):
                            nc.tensor.matmul(
                                ps[m][b][:],
                                lhsT=w1g[gi][:, ci, m * 128:(m + 1) * 128],
                                rhs=xg[:, ci, b * 512:(b + 1) * 512],
                                start=(c == 0),
                                stop=(c == KC - 1),
                            )
                coff += gk

            # drain psum -> SBUF f32, and per-block BN stats
            for m in range(M1):
                for b in range(NBLK):
                    nc.scalar.activation(
                        out=h1f[:, m, b * 512:(b + 1) * 512],
                        in_=ps[m][b][:], func=F.Copy)
                    nc.vector.bn_stats(out=stats1[:, m, b, :], in_=ps[m][b][:])
                nc.vector.bn_aggr(out=pay1[:, m, :], in_=stats1[:, m, :, :])

            # payload = (mean, var + mean^2) per feature
            nc.vector.tensor_tensor(out=t1a[:], in0=pay1[:, :, 0],
                                    in1=pay1[:, :, 0], op=A.mult)
            nc.vector.tensor_tensor(out=pay1[:, :, 1], in0=pay1[:, :, 1],
                                    in1=t1a[:], op=A.add)

            # AllGather (floor ~4.6us vs AllReduce ~10us), reduce locally
            nc.sync.dma_start(out=cin1[:], in_=pay1.rearrange("p m t -> p (m t)"))
            nc.gpsimd.collective_compute(
                "AllGather", A.bypass, replica_groups=RG,
                ins=[cin1.opt()], outs=[cout1.opt()])
            nc.sync.dma_start(out=gath1[:],
                              in_=cout1.rearrange("(r p) t -> p t r", p=128))
            nc.vector.tensor_reduce(out=gst1.rearrange("p m t -> p (m t)"),
                                    in_=gath1[:], axis=mybir.AxisListType.X,
                                    op=A.add)

            # a1 = g1 / sqrt(var + eps), c1 = be1 - mean * a1
            inv = 1.0 / N_CORES
            nc.vector.tensor_scalar(out=gst1[:], in0=gst1[:], scalar1=inv,
                                    scalar2=None, op0=A.mult)  # mean | q
            nc.vector.tensor_tensor(out=t1b[:], in0=gst1[:, :, 0],
                                    in1=gst1[:, :, 0], op=A.mult)       # mean^2
            nc.vector.scalar_tensor_tensor(out=c1[:], in0=gst1[:, :, 1],
                                           scalar=EPS, in1=t1b[:],
                                           op0=A.add, op1=A.subtract)   # var+eps
            nc.scalar.activation(out=c1[:], in_=c1[:], func=F.Sqrt)
            nc.vector.reciprocal(out=t1b[:], in_=c1[:])                 # 1/std
            nc.vector.tensor_tensor(out=a1[:], in0=t1b[:], in1=g1t[:],
                                    op=A.mult)                          # a1
            nc.vector.tensor_tensor(out=t1b[:], in0=gst1[:, :, 0], in1=a1[:],
                                    op=A.mult)                          # mean*a1
            nc.vector.tensor_tensor(out=c1[:], in0=be1t[:], in1=t1b[:],
                                    op=A.subtract)                      # c1

            # ---- phase 2: bn1 + clip + h2 matmul -----------------------------
            # chunk-0 affine block-wise on ScalarE (first matmul starts after
            # one 512-block), chunk-1 affine on VectorE; clips on VectorE
            tmp1a = scr.tile([128, BS], dt.float32, tag="tmp1a")
            tmp1b = scr.tile([128, BS], dt.float32, tag="tmp1b")
            for b in range(NBLK):
                sl = slice(b * 512, (b + 1) * 512)
                nc.scalar.activation(out=tmp1a[:, sl], in_=h1f[:, 0, sl],
                                     func=F.Identity,
                                     bias=c1[:, 0:1], scale=a1[:, 0:1])
                nc.vector.tensor_scalar(out=h1c[:, 0, sl], in0=tmp1a[:, sl],
                                        scalar1=-1.0, scalar2=1.0,
                                        op0=A.max, op1=A.min)
                nc.tensor.matmul(ps[0][b][:], lhsT=w2s[:, 0, :],
                                 rhs=h1c[:, 0, sl], start=True, stop=False)
            nc.vector.tensor_scalar(out=tmp1b[:], in0=h1f[:, 1, :],
                                    scalar1=a1[:, 1:2], scalar2=c1[:, 1:2],
                                    op0=A.mult, op1=A.add)
            for b in range(NBLK):
                sl = slice(b * 512, (b + 1) * 512)
                nc.vector.tensor_scalar(out=h1c[:, 1, sl], in0=tmp1b[:, sl],
                                        scalar1=-1.0, scalar2=1.0,
                                        op0=A.max, op1=A.min)
                nc.tensor.matmul(ps[0][b][:], lhsT=w2s[:, 1, :],
                                 rhs=h1c[:, 1, sl], start=False, stop=True)
                nc.vector.bn_stats(out=stats2[:, b, :], in_=ps[0][b][:])
            nc.vector.bn_aggr(out=pay2[:], in_=stats2[:])

            nc.vector.tensor_tensor(out=t2a[:], in0=pay2[:, 0:1],
                                    in1=pay2[:, 0:1], op=A.mult)
            nc.vector.tensor_tensor(out=pay2[:, 1:2], in0=pay2[:, 1:2],
                                    in1=t2a[:], op=A.add)

            nc.sync.dma_start(out=cin2[:], in_=pay2[:])
            nc.gpsimd.collective_compute(
                "AllGather", A.bypass, replica_groups=RG,
                ins=[cin2.opt()], outs=[cout2.opt()])
            nc.sync.dma_start(out=gath2[:],
                              in_=cout2.rearrange("(r p) t -> p t r", p=128))
            nc.vector.tensor_reduce(out=gst2[:], in_=gath2[:],
                                    axis=mybir.AxisListType.X, op=A.add)

            nc.vector.tensor_scalar(out=gst2[:], in0=gst2[:], scalar1=inv,
                                    scalar2=None, op0=A.mult)  # mean | q
            nc.vector.tensor_tensor(out=t2b[:], in0=gst2[:, 0:1],
                                    in1=gst2[:, 0:1], op=A.mult)
            nc.vector.scalar_tensor_tensor(out=c2[:], in0=gst2[:, 1:2],
                                           scalar=EPS, in1=t2b[:],
                                           op0=A.add, op1=A.subtract)
            nc.scalar.activation(out=c2[:], in_=c2[:], func=F.Sqrt)
            nc.vector.reciprocal(out=t2b[:], in_=c2[:])
            nc.vector.tensor_tensor(out=a2[:], in0=t2b[:], in1=g2t[:],
                                    op=A.mult)
            nc.vector.tensor_tensor(out=t2b[:], in0=gst2[:, 0:1], in1=a2[:],
                                    op=A.mult)
            nc.vector.tensor_tensor(out=c2[:], in0=be2t[:], in1=t2b[:],
                                    op=A.subtract)

            # ---- phase 3: bn2 + clip + final linear --------------------------
            for b in range(NBLK):
                tmp2 = scr.tile([128, 512], dt.float32, tag="tmp2")
                h2c = scr.tile([128, 512], dt.bfloat16, tag="h2c")
                nc.scalar.activation(out=tmp2[:], in_=ps[0][b][:],
                                     func=F.Identity, bias=c2[:], scale=a2[:])
                nc.vector.tensor_scalar(out=h2c[:], in0=tmp2[:],
                                        scalar1=-1.0, scalar2=1.0,
                                        op0=A.max, op1=A.min)
                nc.tensor.matmul(
                    ps[1][b][0:NCOUT, :], lhsT=w4s[:], rhs=h2c[:],
                    start=True, stop=True)
                nc.scalar.activation(out=out_sb[:, b * 512:(b + 1) * 512],
                                     in_=ps[1][b][0:NCOUT, :],
                                     func=F.Identity, bias=b4t[:], scale=1.0)

            nc.sync.dma_start(out=out[:, :], in_=out_sb[:])

    nc.compile()
    return nc


def _get_compiled():
    if "nc" not in _COMPILED:
        _COMPILED["nc"] = _build()
    return _COMPILED["nc"]


def kernel(x, W1, b1, g1, be1, W2, b2, g2, be2, W4, b4, y):
    """Full-input entry point: shards internally across 8 NeuronCores."""
    global LAST_EXEC_NS
    from concourse.bass_utils import run_bass_kernel_spmd

    trace = os.environ.get("BASS_KERNEL_TRACE", "0") == "1"
    if trace:
        _install_ntff_hook()

    x2 = np.asarray(x, dtype=np.float32).reshape(B, IN)
    xb = x2.astype(bf16)

    w1t = np.ascontiguousarray(np.sign(np.asarray(W1, np.float32)).T).astype(bf16)
    w2t = np.ascontiguousarray(np.sign(np.asarray(W2, np.float32)).T).astype(bf16)
    w4t = np.ascontiguousarray(np.asarray(W4, np.float32).T).astype(bf16)
    shared = {
        "w1t": w1t, "w2t": w2t, "w4t": w4t,
        "g1": np.ascontiguousarray(np.asarray(g1, np.float32)),
        "be1": np.ascontiguousarray(np.asarray(be1, np.float32)),
        "g2": np.ascontiguousarray(np.asarray(g2, np.float32)),
        "be2": np.ascontiguousarray(np.asarray(be2, np.float32)),
        "b4": np.ascontiguousarray(np.asarray(b4, np.float32)),
    }
    in_maps = []
    for s in range(N_CORES):
        xts = np.ascontiguousarray(xb[s * BS:(s + 1) * BS, :].T)
        in_maps.append({"xt": xts, **shared})

    nc = _get_compiled()
    res = run_bass_kernel_spmd(nc, in_maps, core_ids=list(range(N_CORES)),
                               trace=trace)
    LAST_EXEC_NS = res.exec_time_ns

    outT = np.concatenate([res.results[s]["out"] for s in range(N_CORES)], axis=1)
    return np.ascontiguousarray(outT.T)


# revision 9
# speedup vs baseline: 1.0693x; 1.0693x over previous
"""Trainium2 Bass kernel for a 3-layer binarized MLP with train-mode BatchNorm.

Network (B=16384, IN=4096, H1=256, H2=128, NC=12):
    h1 = x @ sign(W1).T [+ sign(b1)]        <- bias cancels in train-mode BN
    h1 = clip(BN_train(h1; g1, be1), -1, 1)
    h2 = h1 @ sign(W2).T [+ sign(b2)]       <- bias cancels in train-mode BN
    h2 = clip(BN_train(h2; g2, be2), -1, 1)
    out = h2 @ W4.T + b4

Strategy: pure data-parallel over 8 NeuronCores (2048 batch rows each).
BN statistics are over the FULL batch, so each core computes per-feature
partial (mean, var) via hw bn_stats/bn_aggr and the cores AllReduce a tiny
(mean, var+mean^2) payload between layers.  Everything is computed in a
feature-major layout (features on partitions, batch on the free axis) so
the BN affine is a per-partition scale/bias and batch reductions are
free-axis reductions.

Host-side prep (cheap, linear): fold sign() into the weights, pre-transpose
x and weights so all DMA is contiguous, cast matmul operands to bf16
(measured end-to-end rel-l2 error vs the fp32 reference: ~2.4e-3).
"""

import os
import sys
import types

import numpy as np
import ml_dtypes

B, IN, H1, H2, NCOUT = 16384, 4096, 256, 128, 12
N_CORES = 8
BS = B // N_CORES            # 2048 batch rows per core
NBLK = BS // 512             # 4 psum blocks of 512 along batch
KC = IN // 128               # 32 contraction chunks, layer 1
M1 = H1 // 128               # 2 feature chunks, layer 1 output
GK = 4                       # max k-chunks per x DMA group (2 MB per DMA)
GROUPS = [2, 2] + [4] * 7    # k-chunk group sizes (small first groups
                             # so the first matmuls start early)
assert sum(GROUPS) == KC
EPS = 1e-5

bf16 = ml_dtypes.bfloat16

_COMPILED = {}
LAST_EXEC_NS = None


def _install_ntff_hook():
    """Best-effort: register the axon NTFF profile hook that the boot script
    could not (antenv.axon_hooks is absent from this image)."""
    if "antenv.axon_hooks" in sys.modules:
        return
    try:
        from trn_agent_boot.trn_boot import _ntff_profile_via_ctypes

        hook = _ntff_profile_via_ctypes("/opt/axon/libaxon_pjrt.so")
        mod = types.ModuleType("antenv.axon_hooks")
        mod.get_axon_ntff_profile_hook = lambda: hook
        sys.modules["antenv.axon_hooks"] = mod
    except Exception:
        pass


def _build():
    import concourse.bacc as bacc
    import concourse.tile as tile
    import concourse.mybir as mybir

    dt = mybir.dt
    F = mybir.ActivationFunctionType
    A = mybir.AluOpType
    RG = [list(range(N_CORES))]

    nc = bacc.Bacc("TRN2", target_bir_lowering=False, debug=False,
                   num_devices=N_CORES)

    xt = nc.dram_tensor("xt", [IN, BS], dt.bfloat16, kind="ExternalInput").ap()
    w1t = nc.dram_tensor("w1t", [IN, H1], dt.bfloat16, kind="ExternalInput").ap()
    w2t = nc.dram_tensor("w2t", [H1, H2], dt.bfloat16, kind="ExternalInput").ap()
    w4t = nc.dram_tensor("w4t", [H2, NCOUT], dt.bfloat16, kind="ExternalInput").ap()
    g1 = nc.dram_tensor("g1", [H1], dt.float32, kind="ExternalInput").ap()
    be1 = nc.dram_tensor("be1", [H1], dt.float32, kind="ExternalInput").ap()
    g2 = nc.dram_tensor("g2", [H2], dt.float32, kind="ExternalInput").ap()
    be2 = nc.dram_tensor("be2", [H2], dt.float32, kind="ExternalInput").ap()
    b4 = nc.dram_tensor("b4", [NCOUT], dt.float32, kind="ExternalInput").ap()
    out = nc.dram_tensor("out", [NCOUT, BS], dt.float32, kind="ExternalOutput").ap()

    with tile.TileContext(nc) as tc:
        with (
            tc.tile_pool(name="xp", bufs=3) as xp,
            tc.tile_pool(name="wp", bufs=1) as wp,
            tc.tile_pool(name="hp", bufs=1) as hp,
            tc.tile_pool(name="pp", bufs=1, space="PSUM") as pp,
            tc.tile_pool(name="sp", bufs=1) as sp,
            tc.tile_pool(name="scr", bufs=2) as scr,
            tc.tile_pool(name="dp", bufs=1, space="DRAM") as dp,
        ):
            # ---- static tiles -------------------------------------------------
            w2s = wp.tile([128, M1, H2], dt.bfloat16, tag="w2s")
            w4s = wp.tile([H2, NCOUT], dt.bfloat16, tag="w4s")
            g1t = sp.tile([128, M1], dt.float32, tag="g1t")
            be1t = sp.tile([128, M1], dt.float32, tag="be1t")
            g2t = sp.tile([128, 1], dt.float32, tag="g2t")
            be2t = sp.tile([128, 1], dt.float32, tag="be2t")
            b4t = sp.tile([NCOUT, 1], dt.float32, tag="b4t")

            h1f = hp.tile([128, M1, BS], dt.float32, tag="h1f")
            h1c = hp.tile([128, M1, BS], dt.bfloat16, tag="h1c")
            out_sb = hp.tile([NCOUT, BS], dt.float32, tag="out_sb")

            stats1 = sp.tile([128, M1, NBLK, 6], dt.float32, tag="stats1")
            pay1 = sp.tile([128, M1, 2], dt.float32, tag="pay1")
            gath1 = sp.tile([128, M1 * 2, N_CORES], dt.float32, tag="gath1")
            gst1 = sp.tile([128, M1, 2], dt.float32, tag="gst1")
            a1 = sp.tile([128, M1], dt.float32, tag="a1")
            c1 = sp.tile([128, M1], dt.float32, tag="c1")
            t1a = sp.tile([128, M1], dt.float32, tag="t1a")
            t1b = sp.tile([128, M1], dt.float32, tag="t1b")

            stats2 = sp.tile([128, NBLK, 6], dt.float32, tag="stats2")
            pay2 = sp.tile([128, 2], dt.float32, tag="pay2")
            gath2 = sp.tile([128, 2, N_CORES], dt.float32, tag="gath2")
            gst2 = sp.tile([128, 2], dt.float32, tag="gst2")
            a2 = sp.tile([128, 1], dt.float32, tag="a2")
            c2 = sp.tile([128, 1], dt.float32, tag="c2")
            t2a = sp.tile([128, 1], dt.float32, tag="t2a")
            t2b = sp.tile([128, 1], dt.float32, tag="t2b")

            cin1 = dp.tile([128, M1 * 2], dt.float32, tag="cin1")
            cout1 = dp.tile([128 * N_CORES, M1 * 2], dt.float32, tag="cout1")
            cin2 = dp.tile([128, 2], dt.float32, tag="cin2")
            cout2 = dp.tile([128 * N_CORES, 2], dt.float32, tag="cout2")

            # psum accumulators: 8 banks = (M1 x NBLK) tiles of [128, 512] f32
            ps = [[pp.tile([128, 512], dt.float32, tag=f"ps_{m}_{b}",
                           name=f"ps_{m}_{b}")
                   for b in range(NBLK)] for m in range(M1)]

            # ---- weight / param loads (scalar HWDGE queue, parallel to x) ----
            # w1 arrives in per-group pieces so the first matmuls start early
            w1g = []
            coff = 0
            for gi, gk in enumerate(GROUPS):
                t = wp.tile([128, gk, H1], dt.bfloat16, tag=f"w1g_{gi}",
                            name=f"w1g_{gi}")
                nc.scalar.dma_start(
                    out=t[:],
                    in_=w1t[coff * 128:(coff + gk) * 128, :]
                        .rearrange("(c p) m -> p c m", p=128),
                )
                w1g.append(t)
                coff += gk
            nc.scalar.dma_start(out=w2s[:], in_=w2t.rearrange("(c p) m -> p c m", p=128))
            nc.scalar.dma_start(out=w4s[:], in_=w4t[:, :])
            nc.scalar.dma_start(out=g1t[:], in_=g1.rearrange("(m p) -> p m", p=128))
            nc.scalar.dma_start(out=be1t[:], in_=be1.rearrange("(m p) -> p m", p=128))
            nc.scalar.dma_start(out=g2t[:], in_=g2.rearrange("(p o) -> p o", o=1))
            nc.scalar.dma_start(out=be2t[:], in_=be2.rearrange("(p o) -> p o", o=1))
            nc.scalar.dma_start(out=b4t[:], in_=b4.rearrange("(p o) -> p o", o=1))

            # ---- phase 1: h1 = x @ sign(W1).T  (feature-major) ---------------
            coff = 0
            for gi, gk in enumerate(GROUPS):
                xg = xp.tile([128, GK, BS], dt.bfloat16, tag="xg", name="xg")
                nc.sync.dma_start(
                    out=xg[:, 0:gk, :],
                    in_=xt[coff * 128:(coff + gk) * 128, :]
                        .rearrange("(c p) b -> p c b", p=128),
                )
                for ci in range(gk):
                    c = coff + ci
                    for m in range(M1):
                        for b in range(NBLK):
                            nc.tensor.matmul(
                                ps[m][b][:],
                                lhsT=w1g[gi][:, ci, m * 128:(m + 1) * 128],
                                rhs=xg[:, ci, b * 512:(b + 1) * 512],
                                start=(c == 0),
                                stop=(c == KC - 1),
                            )
                coff += gk

            # drain psum -> SBUF f32, and per-block BN stats
            for m in range(M1):
                for b in range(NBLK):
                    nc.scalar.activation(
                        out=h1f[:, m, b * 512:(b + 1) * 512],
                        in_=ps[m][b][:], func=F.Copy)
                    nc.vector.bn_stats(out=stats1[:, m, b, :], in_=ps[m][b][:])
                nc.vector.bn_aggr(out=pay1[:, m, :], in_=stats1[:, m, :, :])

            # payload = (mean, var + mean^2) per feature
            nc.vector.tensor_tensor(out=t1a[:], in0=pay1[:, :, 0],
                                    in1=pay1[:, :, 0], op=A.mult)
            nc.vector.tensor_tensor(out=pay1[:, :, 1], in0=pay1[:, :, 1],
                                    in1=t1a[:], op=A.add)

            # AllGather (floor ~4.6us vs AllReduce ~10us), reduce locally
            nc.sync.dma_start(out=cin1[:], in_=pay1.rearrange("p m t -> p (m t)"))
            nc.gpsimd.collective_compute(
                "AllGather", A.bypass, replica_groups=RG,
                ins=[cin1.opt()], outs=[cout1.opt()])
            nc.sync.dma_start(out=gath1[:],
                              in_=cout1.rearrange("(r p) t -> p t r", p=128))
            nc.vector.tensor_reduce(out=gst1.rearrange("p m t -> p (m t)"),
                                    in_=gath1[:], axis=mybir.AxisListType.X,
                                    op=A.add)

            # a1 = g1 / sqrt(var + eps), c1 = be1 - mean * a1
            inv = 1.0 / N_CORES
            nc.vector.tensor_scalar(out=gst1[:], in0=gst1[:], scalar1=inv,
                                    scalar2=None, op0=A.mult)  # mean | q
            nc.vector.tensor_tensor(out=t1b[:], in0=gst1[:, :, 0],
                                    in1=gst1[:, :, 0], op=A.mult)       # mean^2
            nc.vector.scalar_tensor_tensor(out=c1[:], in0=gst1[:, :, 1],
                                           scalar=EPS, in1=t1b[:],
                                           op0=A.add, op1=A.subtract)   # var+eps
            nc.scalar.activation(out=c1[:], in_=c1[:], func=F.Sqrt)
            nc.vector.reciprocal(out=t1b[:], in_=c1[:])                 # 1/std
            nc.vector.tensor_tensor(out=a1[:], in0=t1b[:], in1=g1t[:],
                                    op=A.mult)                          # a1
            nc.vector.tensor_tensor(out=t1b[:], in0=gst1[:, :, 0], in1=a1[:],
                                    op=A.mult)                          # mean*a1
            nc.vector.tensor_tensor(out=c1[:], in0=be1t[:], in1=t1b[:],
                                    op=A.subtract)                      # c1

            # ---- phase 2: bn1 + clip + h2 matmul -----------------------------
            # affine writes bf16 directly (ScalarE chunk 0 / VectorE chunk 1),
            # clip runs in place at DVE 4x bf16 rate
            for b in range(NBLK):
                sl = slice(b * 512, (b + 1) * 512)
                nc.scalar.activation(out=h1c[:, 0, sl], in_=h1f[:, 0, sl],
                                     func=F.Identity,
                                     bias=c1[:, 0:1], scale=a1[:, 0:1])
                nc.vector.tensor_scalar(out=h1c[:, 0, sl], in0=h1c[:, 0, sl],
                                        scalar1=-1.0, scalar2=1.0,
                                        op0=A.max, op1=A.min)
                nc.tensor.matmul(ps[0][b][:], lhsT=w2s[:, 0, :],
                                 rhs=h1c[:, 0, sl], start=True, stop=False)
            nc.vector.tensor_scalar(out=h1c[:, 1, :], in0=h1f[:, 1, :],
                                    scalar1=a1[:, 1:2], scalar2=c1[:, 1:2],
                                    op0=A.mult, op1=A.add)
            for b in range(NBLK):
                sl = slice(b * 512, (b + 1) * 512)
                nc.vector.tensor_scalar(out=h1c[:, 1, sl], in0=h1c[:, 1, sl],
                                        scalar1=-1.0, scalar2=1.0,
                                        op0=A.max, op1=A.min)
                nc.tensor.matmul(ps[0][b][:], lhsT=w2s[:, 1, :],
                                 rhs=h1c[:, 1, sl], start=False, stop=True)
                nc.vector.bn_stats(out=stats2[:, b, :], in_=ps[0][b][:])
            nc.vector.bn_aggr(out=pay2[:], in_=stats2[:])

            nc.vector.tensor_tensor(out=t2a[:], in0=pay2[:, 0:1],
                                    in1=pay2[:, 0:1], op=A.mult)
            nc.vector.tensor_tensor(out=pay2[:, 1:2], in0=pay2[:, 1:2],
                                    in1=t2a[:], op=A.add)

            nc.sync.dma_start(out=cin2[:], in_=pay2[:])
            nc.gpsimd.collective_compute(
                "AllGather", A.bypass, replica_groups=RG,
                ins=[cin2.opt()], outs=[cout2.opt()])
            nc.sync.dma_start(out=gath2[:],
                              in_=cout2.rearrange("(r p) t -> p t r", p=128))
            nc.vector.tensor_reduce(out=gst2[:], in_=gath2[:],
                                    axis=mybir.AxisListType.X, op=A.add)

            nc.vector.tensor_scalar(out=gst2[:], in0=gst2[:], scalar1=inv,
                                    scalar2=None, op0=A.mult)  # mean | q
            nc.vector.tensor_tensor(out=t2b[:], in0=gst2[:, 0:1],
                                    in1=gst2[:, 0:1], op=A.mult)
            nc.vector.scalar_tensor_tensor(out=c2[:], in0=gst2[:, 1:2],
                                           scalar=EPS, in1=t2b[:],
                                           op0=A.add, op1=A.subtract)
            nc.scalar.activation(out=c2[:], in_=c2[:], func=F.Sqrt)
            nc.vector.reciprocal(out=t2b[:], in_=c2[:])
            nc.vector.tensor_tensor(out=a2[:], in0=t2b[:], in1=g2t[:],
                                    op=A.mult)
            nc.vector.tensor_tensor(out=t2b[:], in0=gst2[:, 0:1], in1=a2[:],
                                    op=A.mult)
            nc.vector.tensor_tensor(out=c2[:], in0=be2t[:], in1=t2b[:],
                                    op=A.subtract)

            # ---- phase 3: bn2 + clip + final linear --------------------------
            for b in range(NBLK):
                h2c = scr.tile([128, 512], dt.bfloat16, tag="h2c")
                nc.scalar.activation(out=h2c[:], in_=ps[0][b][:],
                                     func=F.Identity, bias=c2[:], scale=a2[:])
                nc.vector.tensor_scalar(out=h2c[:], in0=h2c[:],
                                        scalar1=-1.0, scalar2=1.0,
                                        op0=A.max, op1=A.min)
                nc.tensor.matmul(
                    ps[1][b][0:NCOUT, :], lhsT=w4s[:], rhs=h2c[:],
                    start=True, stop=True)
                nc.scalar.activation(out=out_sb[:, b * 512:(b + 1) * 512],
                                     in_=ps[1][b][0:NCOUT, :],
                                     func=F.Identity, bias=b4t[:], scale=1.0)

            nc.sync.dma_start(out=out[:, :], in_=out_sb[:])

    nc.compile()
    return nc


def _get_compiled():
    if "nc" not in _COMPILED:
        _COMPILED["nc"] = _build()
    return _COMPILED["nc"]


def kernel(x, W1, b1, g1, be1, W2, b2, g2, be2, W4, b4, y):
    """Full-input entry point: shards internally across 8 NeuronCores."""
    global LAST_EXEC_NS
    from concourse.bass_utils import run_bass_kernel_spmd

    trace = os.environ.get("BASS_KERNEL_TRACE", "0") == "1"
    if trace:
        _install_ntff_hook()

    x2 = np.asarray(x, dtype=np.float32).reshape(B, IN)
    xb = x2.astype(bf16)

    w1t = np.ascontiguousarray(np.sign(np.asarray(W1, np.float32)).T).astype(bf16)
    w2t = np.ascontiguousarray(np.sign(np.asarray(W2, np.float32)).T).astype(bf16)
    w4t = np.ascontiguousarray(np.asarray(W4, np.float32).T).astype(bf16)
    shared = {
        "w1t": w1t, "w2t": w2t, "w4t": w4t,
        "g1": np.ascontiguousarray(np.asarray(g1, np.float32)),
        "be1": np.ascontiguousarray(np.asarray(be1, np.float32)),
        "g2": np.ascontiguousarray(np.asarray(g2, np.float32)),
        "be2": np.ascontiguousarray(np.asarray(be2, np.float32)),
        "b4": np.ascontiguousarray(np.asarray(b4, np.float32)),
    }
    in_maps = []
    for s in range(N_CORES):
        xts = np.ascontiguousarray(xb[s * BS:(s + 1) * BS, :].T)
        in_maps.append({"xt": xts, **shared})

    nc = _get_compiled()
    res = run_bass_kernel_spmd(nc, in_maps, core_ids=list(range(N_CORES)),
                               trace=trace)
    LAST_EXEC_NS = res.exec_time_ns

    outT = np.concatenate([res.results[s]["out"] for s in range(N_CORES)], axis=1)
    return np.ascontiguousarray(outT.T)
